# revision 19
# baseline (speedup 1.0000x reference)
"""GCK 3x3 conv layer (nn_GCK3x3Layer) as a Trainium2 Bass kernel on 8 NeuronCores.

Math: out[o,h,w] = sum_{c,r,s} Weff[o,c,r,s] * x[c,h+r,w+s], where Weff is the
GCK linComb folded back through the +/-1 separable basis (done on host in f64).

Sharding: H split across 8 cores (64 output rows each, 66 input rows with halo).

Device scheme ("block-diagonal row-pair packing"): input rows are stored as two
64-partition planes (even rows on partitions 0-63, odd rows on 64-127) at the
same free index, so one K=128 matmul contracts over (channel, row-parity).  Two
consecutive output rows (2m, 2m+1) share one PSUM bank as the two 64-partition
halves of M=128.  Per output-row pair, 6 accumulating matmuls (2 j-slots x 3
column shifts) with block-structured [128,128] weights cover all 9 taps for
both rows at full 128x128 array utilization (3/4 of weight blocks non-zero).

Compute dtype: float32r (single-pass fp32 matmul, ~1.4e-4 rel err, full PE rate
at N>=256 vs 1/4 rate for exact fp32).

Schedule (v2): input rows DMA'd in groups of 5 row-pair slots (1.3 MB per
dma_start, 7 DMAs on the Sync HWDGE ring); matmuls ordered weight-major over
groups of 4 row pairs (4 PSUM banks) so each LDWEIGHTS serves 4 matmuls;
outputs staged 2 pairs per [128,1024] SBUF tile and DMA'd on the Scalar HWDGE
ring (separate issue path from input loads).
"""

import numpy as np

import concourse.bass as bass
import concourse.mybir as mybir
import concourse.tile as tile
from concourse import bacc
from concourse.bass_utils import run_bass_kernel_spmd

# Problem constants (hardcoded per contract)
C = 64          # input channels
O = 64          # output channels
H = W = 514     # input spatial
HO = WO = 512   # output spatial
NCORES = 8
ROWS_PER_CORE = HO // NCORES          # 64 output rows
PAIRS = ROWS_PER_CORE // 2            # 32 row pairs
JT = ROWS_PER_CORE // 2 + 1           # 33 input row-pair slots (incl. halo)
# Input DMA groups (row-pair slots per dma_start), ramped so the first matmul
# isn't gated on a large transfer competing with later queue-parallel loads.
XGS = [1, 1, 2, 3, 4, 5, 5, 5, 4, 3]  # sums to 33
XGO = [sum(XGS[:i]) for i in range(len(XGS))]
# Pair groups sharing one LDWEIGHTS per block-weight (PSUM banks per group),
# ramped to match input-group availability.
PGS = [1, 2] + [4] * 7 + [1]          # sums to 32
NWARM = 8                             # dummy matmuls to lift the PE HAM clock
                                      # gate (1.2->2.4 GHz) during input DMA

V = np.array([[1.0, 1.0, 1.0], [1.0, -1.0, 1.0], [1.0, 1.0, -1.0]], dtype=np.float64)

MM_DT = mybir.dt.float32r   # matmul operand dtype


def _fold_weights(linCombs: np.ndarray) -> np.ndarray:
    """linCombs (O, C*9) -> effective conv kernels Weff (O, C, 3, 3), f64."""
    L = linCombs.astype(np.float64).reshape(O, C, 3, 3)  # k = c*9 + 3i + j
    return np.einsum("ocij,ir,js->ocrs", L, V, V)


def _build_block_weights(Weff: np.ndarray) -> np.ndarray:
    """Build the 6 block lhsT mats, returned as [128, 6, 128] (k, idx, mu).

    idx = 2*s + ab  (ab: 0 = j=m slot "A", 1 = j=m+1 slot "B").
    lhsT[k, mu] so that out[mu, n] = sum_k lhsT[k, mu] * rhs[k, n]:
      k    < 64: even-row plane, channel c=k ; k  >= 64: odd plane, c=k-64
      mu   < 64: out row 2m, o=mu            ; mu >= 64: out row 2m+1, o=mu-64
    """
    Wt = {(r, s): Weff[:, :, r, s].T for r in range(3) for s in range(3)}  # [c, o]
    mats = np.zeros((6, 128, 128), dtype=np.float64)
    for s in range(3):
        A = mats[2 * s]
        A[0:64, 0:64] = Wt[(0, s)]      # even row 2m   -> out 2m   (r=0)
        A[64:128, 0:64] = Wt[(1, s)]    # odd  row 2m+1 -> out 2m   (r=1)
        A[64:128, 64:128] = Wt[(0, s)]  # odd  row 2m+1 -> out 2m+1 (r=0)
        B = mats[2 * s + 1]
        B[0:64, 0:64] = Wt[(2, s)]      # even row 2m+2 -> out 2m   (r=2)
        B[0:64, 64:128] = Wt[(1, s)]    # even row 2m+2 -> out 2m+1 (r=1)
        B[64:128, 64:128] = Wt[(2, s)]  # odd  row 2m+3 -> out 2m+1 (r=2)
    return np.ascontiguousarray(mats.transpose(1, 0, 2).astype(np.float32))


def _build_program():
    nc = bacc.Bacc(None, target_bir_lowering=False)
    xin = nc.declare_dram_parameter("xin", [128, JT, W], MM_DT, isOutput=False)
    wts = nc.declare_dram_parameter("wts", [128, 6, 128], MM_DT, isOutput=False)
    yout = nc.declare_dram_parameter(
        "yout", [PAIRS // 2, 128, 2 * WO], mybir.dt.float32, isOutput=True
    )

    with tile.TileContext(nc) as tc:
        with (
            tc.tile_pool(name="wpool", bufs=1) as wpool,
            tc.tile_pool(name="xpool", bufs=1) as xpool,
            tc.tile_pool(name="opool", bufs=4) as opool,
            tc.tile_pool(name="pspool", bufs=7, space="PSUM") as pspool,
        ):
            wt = wpool.tile([128, 6, 128], MM_DT)
            nc.scalar.dma_start(wt[:], wts[:])

            # PE warmup: garbage matmuls on a zeroed scratch tile while the
            # input DMAs are in flight, so the HAM clock gate is already at
            # 8/8 (2.4 GHz) when the real matmuls start.
            warm = wpool.tile([128, WO], mybir.dt.bfloat16, name="warm")
            nc.gpsimd.memset(warm[:], 0.0)
            wps = pspool.tile(
                [128, WO], mybir.dt.float32, tag="wps", name="wps", bufs=1
            )
            for _ in range(NWARM):
                nc.tensor.matmul(
                    wps[:], warm[:, :128], warm[:], start=True, stop=True
                )

            xgs = []
            for g, n in enumerate(XGS):
                xt = xpool.tile([128, n * W], MM_DT, tag=f"xt{g}", name=f"xt{g}")
                nc.sync.dma_start(xt[:], xin[:, XGO[g] : XGO[g] + n, :])
                xgs.append(xt)

            def rhs(j, s):
                for g in reversed(range(len(XGS))):
                    if j >= XGO[g]:
                        off = j - XGO[g]
                        return xgs[g][:, off * W + s : off * W + s + WO]
                raise AssertionError(j)

            # (pair m) -> its PSUM tile; copies/stores emitted 2 pairs at a time
            pend = {}
            g0 = 0
            for pg in PGS:
                pss = []
                for m in range(g0, g0 + pg):
                    pss.append(
                        pspool.tile([128, WO], mybir.dt.float32, tag="ps", name=f"ps{m}")
                    )
                for idx in range(6):
                    s, ab = idx // 2, idx % 2
                    for mi, m in enumerate(range(g0, g0 + pg)):
                        nc.tensor.matmul(
                            pss[mi][:],
                            wt[:, idx, :],
                            rhs(m + ab, s),
                            start=(idx == 0),
                            stop=(idx == 5),
                        )
                for mi, m in enumerate(range(g0, g0 + pg)):
                    pend[m] = pss[mi]
                    if m % 2 == 1:
                        t = m // 2
                        ot = opool.tile(
                            [128, 2 * WO], mybir.dt.float32, tag="ot", name=f"ot{t}"
                        )
                        nc.vector.tensor_copy(ot[:, :WO], pend[m - 1][:])
                        nc.vector.tensor_copy(ot[:, WO:], pend[m][:])
                        nc.scalar.dma_start(yout[t], ot[:])
                        del pend[m - 1], pend[m]
                g0 += pg

    nc.compile()
    return nc


_NC_CACHE = None


def kernel(input: np.ndarray, linCombs: np.ndarray) -> np.ndarray:
    global _NC_CACHE
    x = np.ascontiguousarray(np.asarray(input, dtype=np.float32))
    L = np.asarray(linCombs, dtype=np.float32)
    assert x.shape == (1, C, H, W), x.shape

    Weff = _fold_weights(L)
    wts_h = _build_block_weights(Weff)

    x0 = x[0]
    in_maps = []
    for core in range(NCORES):
        r0 = core * ROWS_PER_CORE
        ev = x0[:, r0:r0 + 2 * JT:2, :]       # [64, 33, W] even local rows
        od = x0[:, r0 + 1:r0 + 2 * JT:2, :]   # [64, 33, W] odd local rows
        P = np.ascontiguousarray(np.concatenate([ev, od], axis=0))  # [128, 33, W]
        in_maps.append({"xin": P, "wts": wts_h})

    if _NC_CACHE is None:
        _NC_CACHE = _build_program()
    res = run_bass_kernel_spmd(_NC_CACHE, in_maps, list(range(NCORES)))

    out = np.empty((1, O, HO, WO), dtype=np.float32)
    for core in range(NCORES):
        y = res.results[core]["yout"]                     # [16, 128, 1024]
        y = y.reshape(PAIRS // 2, 2, O, 2, WO)            # [t, par, o, u, w]
        y = y.transpose(2, 0, 3, 1, 4).reshape(O, ROWS_PER_CORE, WO)
        r0 = core * ROWS_PER_CORE
        out[0, :, r0:r0 + ROWS_PER_CORE, :] = y
    return out
